# revision 1
# baseline (speedup 1.0000x reference)
"""Cross-attention kernel for Trainium2, 8 NeuronCores, data-parallel over batch.

Problem (per batch element b, one per core):
    q  = x_b @ Wq.T + bq                      [T=1024, C=1024]
    kv = enc_b @ Wkv.T + bkv                  [I=576, 2C]
    per head h (H=16, D=64):
        att = softmax((q_h @ k_h.T) / sqrt(D))
        y_h = att @ v_h
    out = y @ Wo.T + bo                       [T, C]

Design notes:
  - One batch element per core (B=8 == n_cores), no collectives.
  - Weights are pre-transposed on host to [in, out] layout so the
    contraction dim (c) lands on SBUF partitions for matmuls.
  - x / enc are transposed on-device via the PE (out = in.T @ I).
  - Matmuls run as float32r (TF32-like, 1 cyc/row at N>=256) via AP bitcast.
  - Attention is computed in S^T = K_h @ Q_h^T orientation ([i, t]); exp is
    applied without max-subtraction (scores are O(1), exp <= ~e^6).  The
    softmax denominator Z_t falls out of the AV matmul by augmenting V with
    a ones column (lhsT M=65); normalization multiplies y^T by a rank-1
    PE-broadcast of 1/Z.
  - Biases: bq/bk are per-partition adds; bv/bo are rank-1 (K=1) matmul
    accumulates of ones^T (x) bias_row.
"""

import numpy as np

T = 1024
C = 1024
I = 576
H = 16
D = 64
NCC = C // 128          # 8 contraction chunks
NIC = (I + 127) // 128  # 5 i chunks (128,128,128,128,64)
I_CH = [128, 128, 128, 128, 64]
VW = 68                 # per-head column block in V tile: 64 v cols + ones col + pad
SCALE = 1.0 / np.sqrt(D)

_CACHE = {}


def _build_nc():
    import concourse.bass as bass
    import concourse.bacc as bacc
    import concourse.mybir as mybir
    import concourse.tile as tile
    from contextlib import ExitStack

    f32 = mybir.dt.float32
    f32r = mybir.dt.float32r

    nc = bacc.Bacc()

    x_d = nc.dram_tensor("x", [T, C], f32r, kind="ExternalInput")
    enc_d = nc.dram_tensor("enc", [I, C], f32r, kind="ExternalInput")
    wqT_d = nc.dram_tensor("wqT", [C, C], f32r, kind="ExternalInput")
    wkT_d = nc.dram_tensor("wkT", [C, C], f32r, kind="ExternalInput")
    wvT_d = nc.dram_tensor("wvT", [C, C], f32r, kind="ExternalInput")
    woT_d = nc.dram_tensor("woT", [C, C], f32r, kind="ExternalInput")
    bq_d = nc.dram_tensor("bq", [C], f32, kind="ExternalInput")
    bk_d = nc.dram_tensor("bk", [C], f32, kind="ExternalInput")
    bv_d = nc.dram_tensor("bv", [C], f32r, kind="ExternalInput")
    bo_d = nc.dram_tensor("bo", [C], f32r, kind="ExternalInput")
    out_d = nc.dram_tensor("out", [T, C], f32, kind="ExternalOutput")

    with ExitStack() as ctx:
        tc = ctx.enter_context(tile.TileContext(nc))

        # long-lived pools
        resid = ctx.enter_context(tc.tile_pool(name="resid", bufs=1))
        misc = ctx.enter_context(tc.tile_pool(name="misc", bufs=1))
        pa = ctx.enter_context(tc.tile_pool(name="pa", bufs=6, space="PSUM"))
        exps = ctx.enter_context(tc.tile_pool(name="exps", bufs=10))

        # constants (DMA'd from NEFF-embedded data; engines can't memset f32r)
        ident_d = nc.inline_tensor(np.eye(128, dtype=np.float32), name="ident_d")
        ones_d = nc.inline_tensor(np.ones((128, 128), dtype=np.float32), name="ones_d")
        ident = misc.tile([128, 128], f32r)
        nc.sync.dma_start(out=ident, in_=ident_d[:, :].bitcast(f32r))
        ones_t = misc.tile([128, 128], f32r)
        nc.sync.dma_start(out=ones_t, in_=ones_d[:, :].bitcast(f32r))
        bq_t = misc.tile([128, NCC], f32)
        nc.sync.dma_start(out=bq_t, in_=bq_d[:].rearrange("(oc p) -> p oc", p=128))
        bk_t = misc.tile([128, NCC], f32)
        nc.sync.dma_start(out=bk_t, in_=bk_d[:].rearrange("(oc p) -> p oc", p=128))
        bv_row = misc.tile([1, C], f32r)
        nc.sync.dma_start(out=bv_row, in_=bv_d[:].unsqueeze(0))
        bo_row = misc.tile([1, C], f32r)
        nc.sync.dma_start(out=bo_row, in_=bo_d[:].unsqueeze(0))

        # resident tensors
        QT = [resid.tile([128, T], f32r, tag=f"QT{i}", name=f"QT{i}") for i in range(NCC)]
        KT = [resid.tile([128, I], f32r, tag=f"KT{i}", name=f"KT{i}") for i in range(NCC)]
        V3 = [resid.tile([128, H, VW], f32r, tag=f"V{i}", name=f"V{i}") for i in range(NIC)]
        YT = [resid.tile([128, T], f32r, tag=f"YT{i}", name=f"YT{i}") for i in range(NCC)]

        with tc.tile_pool(name="ph1", bufs=1) as ph1, \
             tc.tile_pool(name="xin", bufs=3) as xin, \
             tc.tile_pool(name="wsm", bufs=4) as wsm, \
             tc.tile_pool(name="wv8", bufs=1) as wv8, \
             tc.tile_pool(name="pt", bufs=2, space="PSUM") as pt:

            # ---- enc^T (resident through V proj) ----
            encT = [ph1.tile([128, I], f32r, tag=f"encT{i}", name=f"encT{i}") for i in range(NCC)]
            for ii in range(NIC):
                pi = I_CH[ii]
                e_nat = xin.tile([128, C], f32r, tag="xin")
                nc.sync.dma_start(out=e_nat[:pi], in_=enc_d[ii * 128 : ii * 128 + pi])
                for cc in range(NCC):
                    ps = pt.tile([128, 128], f32r, tag="pt")
                    nc.tensor.transpose(
                        ps[:128, :pi],
                        e_nat[:pi, cc * 128 : (cc + 1) * 128],
                        ident[:pi, :pi],
                    )
                    nc.vector.tensor_copy(
                        encT[cc][:, ii * 128 : ii * 128 + pi], ps[:128, :pi]
                    )

            # ---- x^T in t-halves + Q^T projection ----
            for tch in range(2):
                xTh = [ph1.tile([128, 512], f32r, tag=f"xTh{i}", name=f"xTh{i}") for i in range(NCC)]
                for ts in range(4):
                    tt = tch * 4 + ts
                    x_nat = xin.tile([128, C], f32r, tag="xin")
                    nc.sync.dma_start(out=x_nat, in_=x_d[tt * 128 : (tt + 1) * 128])
                    for cc in range(NCC):
                        ps = pt.tile([128, 128], f32r, tag="pt")
                        nc.tensor.transpose(
                            ps, x_nat[:, cc * 128 : (cc + 1) * 128], ident
                        )
                        nc.vector.tensor_copy(
                            xTh[cc][:, ts * 128 : (ts + 1) * 128], ps
                        )
                # Q^T[o, t-half] = (WqT).T @ x^T ; accumulate over c chunks
                for oc in range(NCC):
                    pq = pa.tile([128, 512], f32, tag="pa")
                    for cc in range(NCC):
                        wch = wsm.tile([128, 128], f32r, tag="wsm")
                        nc.sync.dma_start(
                            out=wch,
                            in_=wqT_d[
                                cc * 128 : (cc + 1) * 128, oc * 128 : (oc + 1) * 128
                            ],
                        )
                        nc.tensor.matmul(
                            pq,
                            wch,
                            xTh[cc],
                            start=(cc == 0),
                            stop=(cc == NCC - 1),
                        )
                    nc.vector.tensor_scalar_add(
                        QT[oc][:, tch * 512 : (tch + 1) * 512],
                        pq,
                        bq_t[:, oc : oc + 1],
                    )

            # ---- K^T projection (i in halves of 288) ----
            for oc in range(NCC):
                pk = [pa.tile([128, 288], f32, tag="pa", name=f"pk{_}") for _ in range(2)]
                for cc in range(NCC):
                    wch = wsm.tile([128, 128], f32r, tag="wsm")
                    nc.sync.dma_start(
                        out=wch,
                        in_=wkT_d[
                            cc * 128 : (cc + 1) * 128, oc * 128 : (oc + 1) * 128
                        ],
                    )
                    for ih in range(2):
                        nc.tensor.matmul(
                            pk[ih],
                            wch,
                            encT[cc][:, ih * 288 : (ih + 1) * 288],
                            start=(cc == 0),
                            stop=(cc == NCC - 1),
                        )
                for ih in range(2):
                    nc.vector.tensor_scalar_add(
                        KT[oc][:, ih * 288 : (ih + 1) * 288],
                        pk[ih],
                        bk_t[:, oc : oc + 1],
                    )

            # ---- V projection into [128, H, VW] layout with ones columns ----
            for ii in range(NIC):
                # ones column (head-block col 64) for the fused Z row in AV
                nc.sync.dma_start(
                    out=V3[ii][:, :, 64:65],
                    in_=ones_d[:, 0:H].bitcast(f32r).unsqueeze(2),
                )
            for och in range(2):
                wvt = [wv8.tile([128, 512], f32r, tag=f"wv{i}", name=f"wv{i}") for i in range(NCC)]
                for cc in range(NCC):
                    nc.sync.dma_start(
                        out=wvt[cc],
                        in_=wvT_d[cc * 128 : (cc + 1) * 128, och * 512 : (och + 1) * 512],
                    )
                for ii in range(NIC):
                    pi = I_CH[ii]
                    pv = pa.tile([128, 512], f32, tag="pa")
                    for cc in range(NCC):
                        nc.tensor.matmul(
                            pv[:pi],
                            encT[cc][:, ii * 128 : ii * 128 + pi],
                            wvt[cc],
                            start=(cc == 0),
                            stop=False,
                        )
                    # bv: rank-1 ones^T (x) bv_row accumulate
                    nc.tensor.matmul(
                        pv[:pi],
                        ones_t[0:1, :pi],
                        bv_row[0:1, och * 512 : (och + 1) * 512],
                        start=False,
                        stop=True,
                    )
                    dst = V3[ii][:pi, och * 8 : och * 8 + 8, 0:64]
                    nc.vector.tensor_copy(
                        dst, pv[:pi].rearrange("p (h d) -> p h d", d=64)
                    )

        # ---- attention ----
        with tc.tile_pool(name="attn", bufs=3) as attn:
            for h in range(H):
                oc = h // 2
                hb = (h % 2) * 64
                for tch in range(2):
                    tsl = slice(tch * 512, (tch + 1) * 512)
                    # S^T chunks -> exp -> sbuf
                    es = []
                    for ii in range(NIC):
                        pi = I_CH[ii]
                        ps = pa.tile([128, 512], f32, tag="pa")
                        nc.tensor.matmul(
                            ps[:pi],
                            KT[oc][hb : hb + 64, ii * 128 : ii * 128 + pi],
                            QT[oc][hb : hb + 64, tsl],
                            start=True,
                            stop=True,
                        )
                        e = exps.tile([128, 512], f32r, tag="exps")
                        nc.scalar.activation(
                            e[:pi],
                            ps[:pi],
                            mybir.ActivationFunctionType.Exp,
                            scale=float(SCALE),
                        )
                        es.append(e)
                    # y^T (64 rows) and Z (row 64) via V augmented with ones col
                    py = pa.tile([128, 512], f32, tag="pa")
                    for ii in range(NIC):
                        pi = I_CH[ii]
                        nc.tensor.matmul(
                            py[:65],
                            V3[ii][:pi, h, 0:65],
                            es[ii][:pi],
                            start=(ii == 0),
                            stop=(ii == NIC - 1),
                        )
                    # r = 1/Z on partition 64; rank-1 broadcast to [64, 512]
                    rz = attn.tile([128, 512], f32r, tag="rz")
                    with nc.allow_low_precision(reason="1/Z in f32r is fine"):
                        nc.vector.reciprocal(rz[64:65], py[64:65])
                    pb = pa.tile([128, 512], f32, tag="pa")
                    nc.tensor.matmul(
                        pb[:64],
                        ones_t[64:65, 0:64],
                        rz[64:65],
                        start=True,
                        stop=True,
                    )
                    zb = attn.tile([64, 512], f32, tag="zb")
                    nc.vector.tensor_copy(zb, pb[:64])
                    nc.vector.tensor_mul(YT[oc][hb : hb + 64, tsl], py[:64], zb)

        # ---- output projection ----
        with tc.tile_pool(name="wo16", bufs=1) as wo16, \
             tc.tile_pool(name="osb", bufs=3) as osb:
            wot = {}
            for cc in range(NCC):
                for och in range(2):
                    w = wo16.tile([128, 512], f32r, tag=f"wo{cc}_{och}", name=f"wo{cc}_{och}")
                    nc.sync.dma_start(
                        out=w,
                        in_=woT_d[
                            cc * 128 : (cc + 1) * 128, och * 512 : (och + 1) * 512
                        ],
                    )
                    wot[(cc, och)] = w
            for tt in range(8):
                ot = osb.tile([128, C], f32, tag="osb")
                for och in range(2):
                    po = pa.tile([128, 512], f32, tag="pa")
                    for cc in range(NCC):
                        nc.tensor.matmul(
                            po,
                            YT[cc][:, tt * 128 : (tt + 1) * 128],
                            wot[(cc, och)],
                            start=(cc == 0),
                            stop=False,
                        )
                    nc.tensor.matmul(
                        po,
                        ones_t[0:1, 0:128],
                        bo_row[0:1, och * 512 : (och + 1) * 512],
                        start=False,
                        stop=True,
                    )
                    nc.vector.tensor_copy(ot[:, och * 512 : (och + 1) * 512], po)
                nc.sync.dma_start(out=out_d[tt * 128 : (tt + 1) * 128], in_=ot)

    nc.compile()
    return nc


def _get_nc():
    if "nc" not in _CACHE:
        _CACHE["nc"] = _build_nc()
    return _CACHE["nc"]


def _prep_in_maps(x, encoder_output, Wq, bq, Wkv, bkv, Wo, bo):
    f = np.float32
    x = np.asarray(x, f)
    enc = np.asarray(encoder_output, f)
    wqT = np.ascontiguousarray(np.asarray(Wq, f).T)
    wkv = np.asarray(Wkv, f)
    wkT = np.ascontiguousarray(wkv[:C].T)
    wvT = np.ascontiguousarray(wkv[C:].T)
    woT = np.ascontiguousarray(np.asarray(Wo, f).T)
    bq = np.asarray(bq, f)
    bkv = np.asarray(bkv, f)
    bo = np.asarray(bo, f)
    shared = {
        "wqT": wqT, "wkT": wkT, "wvT": wvT, "woT": woT,
        "bq": bq, "bk": np.ascontiguousarray(bkv[:C]),
        "bv": np.ascontiguousarray(bkv[C:]), "bo": bo,
    }
    return [
        dict(shared, x=np.ascontiguousarray(x[b]), enc=np.ascontiguousarray(enc[b]))
        for b in range(x.shape[0])
    ]


def kernel(x, encoder_output, Wq, bq, Wkv, bkv, Wo, bo):
    from concourse.bass_utils import run_bass_kernel_spmd

    nc = _get_nc()
    in_maps = _prep_in_maps(x, encoder_output, Wq, bq, Wkv, bkv, Wo, bo)
    res = run_bass_kernel_spmd(nc, in_maps, list(range(len(in_maps)))).results
    return np.stack([res[b]["out"] for b in range(len(res))]).astype(np.float32)



# revision 17
# speedup vs baseline: 1.3829x; 1.3829x over previous
"""Cross-attention kernel for Trainium2, 8 NeuronCores, data-parallel over batch.

Problem (per batch element b, one per core):
    q  = x_b @ Wq.T + bq                      [T=1024, C=1024]
    kv = enc_b @ Wkv.T + bkv                  [I=576, 2C]
    per head h (H=16, D=64):
        att = softmax((q_h @ k_h.T) / sqrt(D))
        y_h = att @ v_h
    out = y @ Wo.T + bo                       [T, C]

v2 design notes (vs baseline):
  - softmax 1/sqrt(D) folded into Wq/bq on host.
  - Weights DMA'd in [128, 1024] tiles (few, big transfers) ordered so the
    PE rarely waits; wq/wo reuse wk/wv buffer slots (sem-gated rotation).
  - Attention normalization uses reciprocal_approx_fast (DVE) and is
    pipelined one iteration deep: the rank-1 1/Z broadcast for iteration
    j-1 runs between iteration j's S^T and AV matmuls, so the PE never
    idles on the softmax denominator.
  - exp runs on 2-bank PSUM tiles ([*,1024] per ACT op) for i-chunks 0..3;
    the last (64-row) chunk uses a Schraudolph bit-trick exp on DVE
    (bits = round(s*128/ln2 + 16250.5) as int16, bitcast bf16).
  - V and exp(S) are bf16 (AV matmul bf16xbf16); Q/K logit path stays f32r.
  - Q/K bias adds moved to the ACT engine (per-partition bias), transpose
    copies alternate DVE/Pool, so no single engine gates the PE.
  - PE program order interleaves attention(t-half 0) between the two Q
    projection halves to spread ACT exp work.
"""

import numpy as np

T = 1024
C = 1024
I = 576
H = 16
D = 64
NCC = C // 128          # 8 contraction chunks
NIC = (I + 127) // 128  # 5 i chunks (128,128,128,128,64)
I_CH = [128, 128, 128, 128, 64]
VW = 66                 # per-head column block in V tile: 64 v cols + ones col + pad
SCALE = 1.0 / np.sqrt(D)
EXP_A16 = 128.0 / np.log(2.0)   # Schraudolph bf16 scale
EXP_B16 = 16256.0 - 5.5         # 127*2^7 minus tuned shift

_CACHE = {}


def _build_nc():
    import concourse.bass as bass
    import concourse.bacc as bacc
    import concourse.mybir as mybir
    import concourse.tile as tile
    from contextlib import ExitStack

    f32 = mybir.dt.float32
    f32r = mybir.dt.float32r
    bf16 = mybir.dt.bfloat16
    i16 = mybir.dt.int16

    nc = bacc.Bacc()

    x_d = nc.dram_tensor("x", [T, C], f32r, kind="ExternalInput")
    enc_d = nc.dram_tensor("enc", [I, C], f32r, kind="ExternalInput")
    wqT_d = nc.dram_tensor("wqT", [C, C], f32r, kind="ExternalInput")
    wkT_d = nc.dram_tensor("wkT", [C, C], f32r, kind="ExternalInput")
    wvT_d = nc.dram_tensor("wvT", [C, C], f32r, kind="ExternalInput")
    woT_d = nc.dram_tensor("woT", [C, C], f32r, kind="ExternalInput")
    bq_d = nc.dram_tensor("bq", [C], f32, kind="ExternalInput")
    bk_d = nc.dram_tensor("bk", [C], f32, kind="ExternalInput")
    bv_d = nc.dram_tensor("bv", [C], bf16, kind="ExternalInput")
    bo_d = nc.dram_tensor("bo", [C], bf16, kind="ExternalInput")
    out_d = nc.dram_tensor("out", [T, C], f32, kind="ExternalOutput")

    with ExitStack() as ctx:
        tc = ctx.enter_context(tile.TileContext(nc))

        # long-lived pools
        resid = ctx.enter_context(tc.tile_pool(name="resid", bufs=1))
        misc = ctx.enter_context(tc.tile_pool(name="misc", bufs=1))

        # constants
        ident_d = nc.inline_tensor(np.eye(128, dtype=np.float32), name="ident_d")
        ones_d = nc.inline_tensor(np.ones((128, 128), dtype=np.float32), name="ones_d")
        ident = misc.tile([128, 128], f32r)
        nc.sync.dma_start(out=ident, in_=ident_d[:, :].bitcast(f32r))
        ones_t = misc.tile([128, 128], f32r)
        nc.sync.dma_start(out=ones_t, in_=ones_d[:, :].bitcast(f32r))
        ones_bf = misc.tile([1, 128], bf16)
        nc.vector.memset(ones_bf, 1.0)
        bq_t = misc.tile([128, NCC], f32)
        nc.sync.dma_start(out=bq_t, in_=bq_d[:].rearrange("(oc p) -> p oc", p=128))
        bk_t = misc.tile([128, NCC], f32)
        nc.sync.dma_start(out=bk_t, in_=bk_d[:].rearrange("(oc p) -> p oc", p=128))
        bv_row = misc.tile([1, C], bf16)
        nc.sync.dma_start(out=bv_row, in_=bv_d[:].unsqueeze(0))
        bo_row = misc.tile([1, C], bf16)
        nc.sync.dma_start(out=bo_row, in_=bo_d[:].unsqueeze(0))

        # resident tensors
        QT = [resid.tile([128, T], f32r, tag=f"QT{i}", name=f"QT{i}") for i in range(NCC)]
        KT = [resid.tile([128, I], f32r, tag=f"KT{i}", name=f"KT{i}") for i in range(NCC)]
        V3 = [resid.tile([128, H, VW], bf16, tag=f"V{i}", name=f"V{i}") for i in range(NIC)]
        YT = [resid.tile([128, T], f32r, tag=f"YT{i}", name=f"YT{i}") for i in range(NCC)]

        copy_idx = [0]

        def transpose_tiles(src_tile, dst_list, dst_col0, pi, pt):
            """PE-transpose src_tile[:pi, cc-chunks] into dst_list[cc][:, col].
            PSUM->SBUF copies alternate DVE / Pool."""
            for cc in range(NCC):
                ps = pt.tile([128, 128], f32r, tag="pt")
                nc.tensor.transpose(
                    ps[:128, :pi],
                    src_tile[:pi, cc * 128 : (cc + 1) * 128],
                    ident[:pi, :pi],
                )
                if copy_idx[0] % 2 == 0:
                    nc.vector.tensor_copy(
                        dst_list[cc][:, dst_col0 : dst_col0 + pi], ps[:128, :pi]
                    )
                else:
                    nc.scalar.copy(
                        dst_list[cc][:, dst_col0 : dst_col0 + pi], ps[:128, :pi]
                    )
                copy_idx[0] += 1

        # SBUF pools live through phase A + attention; wst also through out-proj
        wst = ctx.enter_context(tc.tile_pool(name="wst", bufs=1))
        xin = ctx.enter_context(tc.tile_pool(name="xin", bufs=1))
        xth = ctx.enter_context(tc.tile_pool(name="xth", bufs=1))

        if True:

            # ---- DMA issue order (SP program order) ----
            enc_nat = []
            for ii in range(NIC):
                pi = I_CH[ii]
                e_nat = xin.tile([128, C], f32r, tag=f"xin{ii % 3}", name=f"enc_nat{ii}")
                nc.sync.dma_start(out=e_nat[:pi], in_=enc_d[ii * 128 : ii * 128 + pi])
                enc_nat.append(e_nat)
            x_nat0 = []
            for ts in range(4):
                xt = xin.tile([128, C], f32r, tag=f"xin{(ts + 1) % 3}", name=f"x_nat0_{ts}")
                nc.sync.dma_start(out=xt, in_=x_d[ts * 128 : (ts + 1) * 128])
                x_nat0.append(xt)
            wk_t = []
            for cc in range(NCC):
                w = wst.tile([128, C], f32r, tag=f"w{cc}", name=f"wk{cc}")
                nc.sync.dma_start(out=w, in_=wkT_d[cc * 128 : (cc + 1) * 128, :])
                wk_t.append(w)
            wv_t = []
            for cc in range(NCC):
                w = wst.tile([128, C], f32r, tag=f"w{8 + cc}", name=f"wv{cc}")
                nc.sync.dma_start(out=w, in_=wvT_d[cc * 128 : (cc + 1) * 128, :])
                wv_t.append(w)
            # wq reuses wk slots (gated on K proj), wo reuses wv slots (gated on V proj)
            wq_t = []
            for cc in range(NCC):
                w = wst.tile([128, C], f32r, tag=f"w{cc}", name=f"wq{cc}")
                nc.sync.dma_start(out=w, in_=wqT_d[cc * 128 : (cc + 1) * 128, :])
                wq_t.append(w)
            x_nat1 = []
            for ts in range(4):
                xt = xin.tile([128, C], f32r, tag=f"xin{(ts + 2) % 3}", name=f"x_nat1_{ts}")
                nc.sync.dma_start(out=xt, in_=x_d[(4 + ts) * 128 : (5 + ts) * 128])
                x_nat1.append(xt)
            wo_t = []
            for cc in range(NCC):
                w = wst.tile([128, C], f32r, tag=f"w{8 + cc}", name=f"wo{cc}")
                nc.sync.dma_start(out=w, in_=woT_d[cc * 128 : (cc + 1) * 128, :])
                wo_t.append(w)

            def q_proj_half(tch, xTh, pa):
                for oc in range(NCC):
                    pq = pa.tile([128, 512], f32, tag="pa")
                    for cc in range(NCC):
                        nc.tensor.matmul(
                            pq,
                            wq_t[cc][:, oc * 128 : (oc + 1) * 128],
                            xTh[cc],
                            start=(cc == 0),
                            stop=(cc == NCC - 1),
                        )
                    nc.scalar.add(
                        QT[oc][:, tch * 512 : (tch + 1) * 512],
                        pq,
                        bq_t[:, oc : oc + 1],
                    )

            xTh0 = [xth.tile([128, 512], f32r, tag=f"xTh{i}", name=f"xTh0_{i}") for i in range(NCC)]

            with tc.tile_pool(name="ph1", bufs=1) as ph1, \
                 tc.tile_pool(name="pt1", bufs=3, space="PSUM") as pt, \
                 tc.tile_pool(name="pa1", bufs=4, space="PSUM") as pa:

                encT = [ph1.tile([128, I], f32r, tag=f"encT{i}", name=f"encT{i}") for i in range(NCC)]

                # ---- enc^T, x^T half 0 ----
                for ii in range(NIC):
                    transpose_tiles(enc_nat[ii], encT, ii * 128, I_CH[ii], pt)
                for ts in range(4):
                    transpose_tiles(x_nat0[ts], xTh0, ts * 128, 128, pt)

                # ---- K^T projection (i in halves of 288); bias on ACT ----
                for oc in range(NCC):
                    pk = [pa.tile([128, 288], f32, tag="pa", name=f"pk{_}") for _ in range(2)]
                    for cc in range(NCC):
                        for ih in range(2):
                            nc.tensor.matmul(
                                pk[ih],
                                wk_t[cc][:, oc * 128 : (oc + 1) * 128],
                                encT[cc][:, ih * 288 : (ih + 1) * 288],
                                start=(cc == 0),
                                stop=(cc == NCC - 1),
                            )
                    for ih in range(2):
                        nc.scalar.add(
                            KT[oc][:, ih * 288 : (ih + 1) * 288],
                            pk[ih],
                            bk_t[:, oc : oc + 1],
                        )

                # ---- V projection into [128, H, VW] bf16 layout ----
                for ii in range(NIC):
                    nc.vector.memset(V3[ii][:, :, 64:65], 1.0)
                for och in range(2):
                    for ii in range(NIC):
                        pi = I_CH[ii]
                        pv = pa.tile([128, 512], f32, tag="pa")
                        for cc in range(NCC):
                            nc.tensor.matmul(
                                pv[:pi],
                                encT[cc][:, ii * 128 : ii * 128 + pi],
                                wv_t[cc][:, och * 512 : (och + 1) * 512],
                                start=(cc == 0),
                                stop=False,
                            )
                        nc.tensor.matmul(
                            pv[:pi],
                            ones_bf[0:1, :pi],
                            bv_row[0:1, och * 512 : (och + 1) * 512],
                            start=False,
                            stop=True,
                        )
                        dst = V3[ii][:pi, och * 8 : och * 8 + 8, 0:64]
                        nc.vector.tensor_copy(
                            dst, pv[:pi].rearrange("p (h d) -> p h d", d=64)
                        )

                # ---- Q^T projection half 0 ----
                q_proj_half(0, xTh0, pa)

            def attention_half(tch):
                with tc.tile_pool(name=f"es{tch}", bufs=3) as exps, \
                     tc.tile_pool(name=f"zs{tch}", bufs=1) as zsb, \
                     tc.tile_pool(name=f"rr{tch}", bufs=1) as rrp, \
                     tc.tile_pool(name=f"rp{tch}", bufs=2) as rzp, \
                     tc.tile_pool(name=f"p2{tch}", bufs=2, space="PSUM") as pes2, \
                     tc.tile_pool(name=f"p1{tch}", bufs=1, space="PSUM") as pes1, \
                     tc.tile_pool(name=f"py{tch}", bufs=2, space="PSUM") as pys, \
                     tc.tile_pool(name=f"pz{tch}", bufs=1, space="PSUM") as pzs:
                    tsl = slice(tch * 512, (tch + 1) * 512)
                    norm_idx = [0]

                    def finish_norm(p):
                        py_p, rz_p, oc_p, hb_p = p
                        pb = pzs.tile([128, 512], f32, tag="pz")
                        nc.tensor.matmul(
                            pb[:64],
                            ones_t[64:65, 0:64],
                            rz_p[64:65],
                            start=True,
                            stop=True,
                        )
                        zb = zsb.tile([64, 512], f32, tag="zb")
                        if norm_idx[0] % 2 == 0:
                            nc.scalar.copy(zb, pb[:64])
                        else:
                            nc.vector.tensor_copy(zb, pb[:64])
                        norm_idx[0] += 1
                        nc.vector.tensor_mul(
                            YT[oc_p][hb_p : hb_p + 64, tsl], py_p[:64], zb
                        )

                    pend = None  # (py, rz, oc, hb) of previous iteration
                    for h in range(H):
                        oc = h // 2
                        hb = (h % 2) * 64
                        # S^T chunk pairs into 2-bank psum tiles
                        pAB = [pes2.tile([128, 1024], f32, tag="pe2", name=f"pAB{_}") for _ in range(2)]
                        pC = pes1.tile([128, 512], f32, tag="pe1")
                        for ii in range(NIC):
                            pi = I_CH[ii]
                            dst = pC[:pi, 0:512] if ii == 4 else \
                                pAB[ii // 2][:pi, (ii % 2) * 512 : (ii % 2 + 1) * 512]
                            nc.tensor.matmul(
                                dst,
                                KT[oc][hb : hb + 64, ii * 128 : ii * 128 + pi],
                                QT[oc][hb : hb + 64, tsl],
                                start=True,
                                stop=True,
                            )
                        # exp: chunks 0..3 exact on ACT (1024-wide), chunk 4 via
                        # Schraudolph bit-trick on DVE (bf16 out)
                        eAB = [exps.tile([128, 1024], bf16, tag="eL", name=f"eAB{_}") for _ in range(2)]
                        eC = exps.tile([128, 512], bf16, tag="eS")
                        for j in range(2):
                            for jh in range(2):
                                sl = slice(jh * 512, (jh + 1) * 512)
                                nc.scalar.activation(
                                    eAB[j][:, sl], pAB[j][:, sl],
                                    mybir.ActivationFunctionType.Exp,
                                )
                        with nc.allow_low_precision(reason="schraudolph exp, last chunk"):
                            nc.vector.tensor_scalar(
                                eC[:64].bitcast(i16),
                                pC[:64, 0:512],
                                float(EXP_A16),
                                float(EXP_B16),
                                mybir.AluOpType.mult,
                                mybir.AluOpType.add,
                            )
                        # normalization for PREVIOUS iteration (PE slot here)
                        if pend is not None:
                            finish_norm(pend)
                        # AV with ones column -> y' rows 0..63, Z row 64
                        py = pys.tile([128, 512], f32, tag="py")
                        for ii in range(NIC):
                            pi = I_CH[ii]
                            src = eC[:pi, 0:512] if ii == 4 else \
                                eAB[ii // 2][:pi, (ii % 2) * 512 : (ii % 2 + 1) * 512]
                            nc.tensor.matmul(
                                py[:65],
                                V3[ii][:pi, h, 0:65],
                                src,
                                start=(ii == 0),
                                stop=(ii == NIC - 1),
                            )
                        # 1/Z on DVE (exact, baseline-style)
                        rz = rzp.tile([128, 512], f32r, tag="rz")
                        with nc.allow_low_precision(reason="1/Z in f32r is fine"):
                            nc.vector.reciprocal(rz[64:65], py[64:65])
                        pend = (py, rz, oc, hb)

                    finish_norm(pend)

            attention_half(0)

            # ---- x^T half 1 + Q^T projection half 1 ----
            xTh1 = [xth.tile([128, 512], f32r, tag=f"xTh{i}", name=f"xTh1_{i}") for i in range(NCC)]
            with tc.tile_pool(name="pt2", bufs=3, space="PSUM") as pt, \
                 tc.tile_pool(name="pa2", bufs=3, space="PSUM") as pa:
                for ts in range(4):
                    transpose_tiles(x_nat1[ts], xTh1, ts * 128, 128, pt)
                q_proj_half(1, xTh1, pa)

            attention_half(1)

        # ---- output projection ----
        with tc.tile_pool(name="osb", bufs=3) as osb, \
             tc.tile_pool(name="po", bufs=4, space="PSUM") as pos:
            for tt in range(8):
                ot = osb.tile([128, C], f32, tag="osb")
                for och in range(2):
                    po = pos.tile([128, 512], f32, tag="po")
                    for cc in range(NCC):
                        nc.tensor.matmul(
                            po,
                            YT[cc][:, tt * 128 : (tt + 1) * 128],
                            wo_t[cc][:, och * 512 : (och + 1) * 512],
                            start=(cc == 0),
                            stop=False,
                        )
                    nc.tensor.matmul(
                        po,
                        ones_bf[0:1, 0:128],
                        bo_row[0:1, och * 512 : (och + 1) * 512],
                        start=False,
                        stop=True,
                    )
                    if och == 0:
                        nc.scalar.copy(ot[:, 0:512], po)
                    else:
                        nc.vector.tensor_copy(ot[:, 512:1024], po)
                nc.sync.dma_start(out=out_d[tt * 128 : (tt + 1) * 128], in_=ot)

    nc.compile()
    return nc


def _get_nc():
    if "nc" not in _CACHE:
        _CACHE["nc"] = _build_nc()
    return _CACHE["nc"]


def _prep_in_maps(x, encoder_output, Wq, bq, Wkv, bkv, Wo, bo):
    import ml_dtypes

    f = np.float32
    x = np.asarray(x, f)
    enc = np.asarray(encoder_output, f)
    # fold softmax 1/sqrt(D) into Wq/bq
    wqT = np.ascontiguousarray(np.asarray(Wq, f).T * np.float32(SCALE))
    wkv = np.asarray(Wkv, f)
    wkT = np.ascontiguousarray(wkv[:C].T)
    wvT = np.ascontiguousarray(wkv[C:].T)
    woT = np.ascontiguousarray(np.asarray(Wo, f).T)
    bq = np.asarray(bq, f) * np.float32(SCALE)
    bkv = np.asarray(bkv, f)
    bo = np.asarray(bo, f)
    shared = {
        "wqT": wqT, "wkT": wkT, "wvT": wvT, "woT": woT,
        "bq": bq, "bk": np.ascontiguousarray(bkv[:C]),
        "bv": np.ascontiguousarray(bkv[C:]).astype(ml_dtypes.bfloat16),
        "bo": bo.astype(ml_dtypes.bfloat16),
    }
    return [
        dict(shared, x=np.ascontiguousarray(x[b]), enc=np.ascontiguousarray(enc[b]))
        for b in range(x.shape[0])
    ]


def kernel(x, encoder_output, Wq, bq, Wkv, bkv, Wo, bo):
    from concourse.bass_utils import run_bass_kernel_spmd

    nc = _get_nc()
    in_maps = _prep_in_maps(x, encoder_output, Wq, bq, Wkv, bkv, Wo, bo)
    res = run_bass_kernel_spmd(nc, in_maps, list(range(len(in_maps)))).results
    return np.stack([res[b]["out"] for b in range(len(res))]).astype(np.float32)


# revision 29
# speedup vs baseline: 1.5187x; 1.0981x over previous
"""Cross-attention kernel for Trainium2, 8 NeuronCores, data-parallel over batch.

Problem (per batch element b, one per core):
    q  = x_b @ Wq.T + bq                      [T=1024, C=1024]
    kv = enc_b @ Wkv.T + bkv                  [I=576, 2C]
    per head h (H=16, D=64):
        att = softmax((q_h @ k_h.T) / sqrt(D))
        y_h = att @ v_h
    out = y @ Wo.T + bo                       [T, C]

v2 design notes (vs baseline):
  - softmax 1/sqrt(D) folded into Wq/bq on host.
  - Weights DMA'd in [128, 1024] tiles (few, big transfers) ordered so the
    PE rarely waits; wq/wo reuse wk/wv buffer slots (sem-gated rotation).
  - Attention normalization uses reciprocal_approx_fast (DVE) and is
    pipelined one iteration deep: the rank-1 1/Z broadcast for iteration
    j-1 runs between iteration j's S^T and AV matmuls, so the PE never
    idles on the softmax denominator.
  - exp runs on 2-bank PSUM tiles ([*,1024] per ACT op) for i-chunks 0..3;
    the last (64-row) chunk uses a Schraudolph bit-trick exp on DVE
    (bits = round(s*128/ln2 + 16250.5) as int16, bitcast bf16).
  - V and exp(S) are bf16 (AV matmul bf16xbf16); Q/K logit path stays f32r.
  - Q/K bias adds moved to the ACT engine (per-partition bias), transpose
    copies alternate DVE/Pool, so no single engine gates the PE.
  - PE program order interleaves attention(t-half 0) between the two Q
    projection halves to spread ACT exp work.
"""

import numpy as np

T = 1024
C = 1024
I = 576
H = 16
D = 64
NCC = C // 128          # 8 contraction chunks
NIC = (I + 127) // 128  # 5 i chunks (128,128,128,128,64)
I_CH = [128, 128, 128, 128, 64]
VW = 66                 # per-head column block in V tile: 64 v cols + ones col + pad
SCALE = 1.0 / np.sqrt(D)
EXP_A16 = 128.0 / np.log(2.0)   # Schraudolph bf16 scale
EXP_B16 = 16256.0 - 5.5         # 127*2^7 minus tuned shift

_CACHE = {}


def _build_nc():
    import concourse.bass as bass
    import concourse.bacc as bacc
    import concourse.mybir as mybir
    import concourse.tile as tile
    from contextlib import ExitStack

    f32 = mybir.dt.float32
    f32r = mybir.dt.float32r
    bf16 = mybir.dt.bfloat16
    i16 = mybir.dt.int16

    nc = bacc.Bacc()

    x_d = nc.dram_tensor("x", [T, C], f32r, kind="ExternalInput")
    enc_d = nc.dram_tensor("enc", [I, C], f32r, kind="ExternalInput")
    wqT_d = nc.dram_tensor("wqT", [C, C], f32r, kind="ExternalInput")
    wkT_d = nc.dram_tensor("wkT", [C, C], f32r, kind="ExternalInput")
    wvT_d = nc.dram_tensor("wvT", [C, C], f32r, kind="ExternalInput")
    woT_d = nc.dram_tensor("woT", [C, C], f32r, kind="ExternalInput")
    bq_d = nc.dram_tensor("bq", [C], f32, kind="ExternalInput")
    bk_d = nc.dram_tensor("bk", [C], f32, kind="ExternalInput")
    bv_d = nc.dram_tensor("bv", [C], bf16, kind="ExternalInput")
    bo_d = nc.dram_tensor("bo", [C], bf16, kind="ExternalInput")
    out_d = nc.dram_tensor("out", [T, C], f32, kind="ExternalOutput")

    with ExitStack() as ctx:
        tc = ctx.enter_context(tile.TileContext(nc))

        # long-lived pools
        resid = ctx.enter_context(tc.tile_pool(name="resid", bufs=1))
        misc = ctx.enter_context(tc.tile_pool(name="misc", bufs=1))

        # constants
        ident_d = nc.inline_tensor(np.eye(128, dtype=np.float32), name="ident_d")
        _sel = np.zeros((128, 256), dtype=np.float32)
        for _r in range(4):
            _sel[32 * _r, _r * 64 : (_r + 1) * 64] = 1.0
        sel32_d = nc.inline_tensor(_sel, name="sel32_d")
        ident = misc.tile([128, 128], f32r)
        nc.sync.dma_start(out=ident, in_=ident_d[:, :].bitcast(f32r))
        sel32 = misc.tile([128, 256], f32r)
        nc.sync.dma_start(out=sel32, in_=sel32_d[:, :].bitcast(f32r))
        ones_bf = misc.tile([1, 128], bf16)
        nc.vector.memset(ones_bf, 1.0)
        bq_t = misc.tile([128, NCC], f32)
        nc.sync.dma_start(out=bq_t, in_=bq_d[:].rearrange("(oc p) -> p oc", p=128))
        bk_t = misc.tile([128, NCC], f32)
        nc.sync.dma_start(out=bk_t, in_=bk_d[:].rearrange("(oc p) -> p oc", p=128))

        # resident tensors
        QT = [resid.tile([128, T], f32r, tag=f"QT{i}", name=f"QT{i}") for i in range(NCC)]
        KT = [resid.tile([128, I], f32r, tag=f"KT{i}", name=f"KT{i}") for i in range(NCC)]
        V3 = [resid.tile([128, H, VW], bf16, tag=f"V{i}", name=f"V{i}") for i in range(NIC)]
        YT = [resid.tile([128, T], f32r, tag=f"YT{i}", name=f"YT{i}") for i in range(NCC)]

        copy_idx = [0]

        def transpose_tiles(src_tile, dst_list, dst_col0, pi, pt):
            """PE-transpose src_tile[:pi, cc-chunks] into dst_list[cc][:, col].
            PSUM->SBUF copies alternate DVE / Pool."""
            for cc in range(NCC):
                ps = pt.tile([128, 128], f32r, tag="pt")
                nc.tensor.transpose(
                    ps[:128, :pi],
                    src_tile[:pi, cc * 128 : (cc + 1) * 128],
                    ident[:pi, :pi],
                )
                if copy_idx[0] % 2 == 0:
                    nc.vector.tensor_copy(
                        dst_list[cc][:, dst_col0 : dst_col0 + pi], ps[:128, :pi]
                    )
                else:
                    nc.scalar.copy(
                        dst_list[cc][:, dst_col0 : dst_col0 + pi], ps[:128, :pi]
                    )
                copy_idx[0] += 1

        # SBUF pools live through phase A + attention; wst also through out-proj
        wst = ctx.enter_context(tc.tile_pool(name="wst", bufs=1))
        xin = ctx.enter_context(tc.tile_pool(name="xin", bufs=1))
        xth = ctx.enter_context(tc.tile_pool(name="xth", bufs=1))

        if True:

            # ---- DMA issue order (SP program order) ----
            enc_nat = []
            for ii in range(NIC):
                pi = I_CH[ii]
                e_nat = xin.tile([128, C], f32r, tag=f"xin{ii % 3}", name=f"enc_nat{ii}")
                nc.sync.dma_start(out=e_nat[:pi], in_=enc_d[ii * 128 : ii * 128 + pi])
                enc_nat.append(e_nat)
            x_nat0 = []
            for ts in range(4):
                xt = xin.tile([128, C], f32r, tag=f"xin{(ts + 1) % 3}", name=f"x_nat0_{ts}")
                nc.sync.dma_start(out=xt, in_=x_d[ts * 128 : (ts + 1) * 128])
                x_nat0.append(xt)
            wk_t = []
            for cc in range(NCC):
                w = wst.tile([128, C], f32r, tag=f"w{cc}", name=f"wk{cc}")
                nc.sync.dma_start(out=w, in_=wkT_d[cc * 128 : (cc + 1) * 128, :])
                wk_t.append(w)
            wv_t = []
            for cc in range(NCC):
                w = wst.tile([128, C], f32r, tag=f"w{8 + cc}", name=f"wv{cc}")
                nc.sync.dma_start(out=w, in_=wvT_d[cc * 128 : (cc + 1) * 128, :])
                wv_t.append(w)
            # wq reuses wk slots (gated on K proj), wo reuses wv slots (gated on V proj)
            wq_t = []
            for cc in range(NCC):
                w = wst.tile([128, C], f32r, tag=f"w{cc}", name=f"wq{cc}")
                nc.sync.dma_start(out=w, in_=wqT_d[cc * 128 : (cc + 1) * 128, :])
                wq_t.append(w)
            x_nat1 = []
            for ts in range(4):
                xt = xin.tile([128, C], f32r, tag=f"xin{(ts + 2) % 3}", name=f"x_nat1_{ts}")
                nc.sync.dma_start(out=xt, in_=x_d[(4 + ts) * 128 : (5 + ts) * 128])
                x_nat1.append(xt)
            wo_t = []
            for cc in range(NCC):
                w = wst.tile([128, C], f32r, tag=f"w{8 + cc}", name=f"wo{cc}")
                nc.sync.dma_start(out=w, in_=woT_d[cc * 128 : (cc + 1) * 128, :])
                wo_t.append(w)

            def q_proj_half(tch, xTh, pa):
                for oc in range(NCC):
                    pq = pa.tile([128, 512], f32, tag="pa")
                    for cc in range(NCC):
                        nc.tensor.matmul(
                            pq,
                            wq_t[cc][:, oc * 128 : (oc + 1) * 128],
                            xTh[cc],
                            start=(cc == 0),
                            stop=(cc == NCC - 1),
                        )
                    nc.scalar.add(
                        QT[oc][:, tch * 512 : (tch + 1) * 512],
                        pq,
                        bq_t[:, oc : oc + 1],
                    )

            xTh0 = [xth.tile([128, 512], f32r, tag=f"xTh{i}", name=f"xTh0_{i}") for i in range(NCC)]

            with tc.tile_pool(name="ph1", bufs=1) as ph1, \
                 tc.tile_pool(name="pt1", bufs=3, space="PSUM") as pt, \
                 tc.tile_pool(name="pa1", bufs=4, space="PSUM") as pa:

                encT = [ph1.tile([128, I], f32r, tag=f"encT{i}", name=f"encT{i}") for i in range(NCC)]
                bv_row = ph1.tile([1, C], bf16, tag="bv_row")
                nc.sync.dma_start(out=bv_row, in_=bv_d[:].unsqueeze(0))

                # ---- enc^T, x^T half 0 ----
                for ii in range(NIC):
                    transpose_tiles(enc_nat[ii], encT, ii * 128, I_CH[ii], pt)
                for ts in range(4):
                    transpose_tiles(x_nat0[ts], xTh0, ts * 128, 128, pt)

                # ---- K^T projection (i in halves of 288); bias on ACT ----
                for oc in range(NCC):
                    pk = [pa.tile([128, 288], f32, tag="pa", name=f"pk{_}") for _ in range(2)]
                    for cc in range(NCC):
                        for ih in range(2):
                            nc.tensor.matmul(
                                pk[ih],
                                wk_t[cc][:, oc * 128 : (oc + 1) * 128],
                                encT[cc][:, ih * 288 : (ih + 1) * 288],
                                start=(cc == 0),
                                stop=(cc == NCC - 1),
                            )
                    for ih in range(2):
                        nc.scalar.add(
                            KT[oc][:, ih * 288 : (ih + 1) * 288],
                            pk[ih],
                            bk_t[:, oc : oc + 1],
                        )

                # ---- V projection into [128, H, VW] bf16 layout ----
                for ii in range(NIC):
                    nc.vector.memset(V3[ii][:, :, 64:65], 1.0)
                for och in range(2):
                    for ii in range(NIC):
                        pi = I_CH[ii]
                        pv = pa.tile([128, 512], f32, tag="pa")
                        for cc in range(NCC):
                            nc.tensor.matmul(
                                pv[:pi],
                                encT[cc][:, ii * 128 : ii * 128 + pi],
                                wv_t[cc][:, och * 512 : (och + 1) * 512],
                                start=(cc == 0),
                                stop=False,
                            )
                        nc.tensor.matmul(
                            pv[:pi],
                            ones_bf[0:1, :pi],
                            bv_row[0:1, och * 512 : (och + 1) * 512],
                            start=False,
                            stop=True,
                        )
                        dst = V3[ii][:pi, och * 8 : och * 8 + 8, 0:64]
                        nc.vector.tensor_copy(
                            dst, pv[:pi].rearrange("p (h d) -> p h d", d=64)
                        )

                # ---- Q^T projection half 0 ----
                q_proj_half(0, xTh0, pa)

            def attention_half(tch):
                # Normalization: Z rows are gathered into Zbuf[8, 512]; one
                # exact reciprocal per 8 heads (amortized); the 1/Z broadcast
                # (one-hot K=8 matmul) + in-place YT multiply run lagged by
                # one batch so the PE never waits on the reciprocal.
                with tc.tile_pool(name=f"es{tch}", bufs=3) as exps, \
                     tc.tile_pool(name=f"zb{tch}", bufs=1) as zbp, \
                     tc.tile_pool(name=f"zr{tch}", bufs=1) as zrp, \
                     tc.tile_pool(name=f"p2{tch}", bufs=2, space="PSUM") as pes2, \
                     tc.tile_pool(name=f"p1{tch}", bufs=1, space="PSUM") as pes1, \
                     tc.tile_pool(name=f"py{tch}", bufs=2, space="PSUM") as pys, \
                     tc.tile_pool(name=f"pz{tch}", bufs=1, space="PSUM") as pzs:
                    tsl = slice(tch * 512, (tch + 1) * 512)

                    # Z rows for head j=2b+a of a batch of 8 live at partition
                    # 32a, free block b (engine partition bases must be
                    # 32-aligned). Two batches per half; batch-0 norms run
                    # spread across batch-1 iterations, batch-1 norms trail.
                    state = {"Zbuf": None, "Zr": [None, None]}

                    def norm_one(h):
                        j = h % 8
                        a, b = j % 4, j // 4
                        oc_p, hb_p = h // 2, (h % 2) * 64
                        pb = pzs.tile([128, 512], f32, tag="pz")
                        nc.tensor.matmul(
                            pb[:64],
                            sel32[:, a * 64 : (a + 1) * 64],
                            state["Zr"][h // 8][:, 512 * b : 512 * b + 512],
                            start=True,
                            stop=True,
                        )
                        nc.vector.tensor_mul(
                            YT[oc_p][hb_p : hb_p + 64, tsl],
                            YT[oc_p][hb_p : hb_p + 64, tsl],
                            pb[:64],
                        )

                    for h in range(H):
                        oc = h // 2
                        hb = (h % 2) * 64
                        if h % 8 == 0:
                            state["Zbuf"] = zbp.tile(
                                [128, 1024], f32, tag="zbuf", name=f"zbuf{h}"
                            )
                            nc.gpsimd.memset(state["Zbuf"], 1.0)
                        # S^T chunk pairs into 2-bank psum tiles
                        pAB = [pes2.tile([128, 1024], f32, tag="pe2", name=f"pAB{_}") for _ in range(2)]
                        pC = pes1.tile([128, 512], f32, tag="pe1")
                        for ii in range(NIC):
                            pi = I_CH[ii]
                            dst = pC[:pi, 0:512] if ii == 4 else \
                                pAB[ii // 2][:pi, (ii % 2) * 512 : (ii % 2 + 1) * 512]
                            nc.tensor.matmul(
                                dst,
                                KT[oc][hb : hb + 64, ii * 128 : ii * 128 + pi],
                                QT[oc][hb : hb + 64, tsl],
                                start=True,
                                stop=True,
                            )
                        # exp: chunks 0..3 exact on ACT (1024-wide), chunk 4 via
                        # Schraudolph bit-trick on DVE (bf16 out)
                        eAB = [exps.tile([128, 1024], bf16, tag="eL", name=f"eAB{_}") for _ in range(2)]
                        eC = exps.tile([128, 512], bf16, tag="eS")
                        for j in range(2):
                            for jh in range(2):
                                sl = slice(jh * 512, (jh + 1) * 512)
                                nc.scalar.activation(
                                    eAB[j][:, sl], pAB[j][:, sl],
                                    mybir.ActivationFunctionType.Exp,
                                )
                        with nc.allow_low_precision(reason="schraudolph exp, last chunk"):
                            nc.vector.tensor_scalar(
                                eC[:64].bitcast(i16),
                                pC[:64, 0:512],
                                float(EXP_A16),
                                float(EXP_B16),
                                mybir.AluOpType.mult,
                                mybir.AluOpType.add,
                            )
                        # AV with ones column -> y' rows 0..63, Z row 64
                        py = pys.tile([128, 512], f32, tag="py")
                        for ii in range(NIC):
                            pi = I_CH[ii]
                            src = eC[:pi, 0:512] if ii == 4 else \
                                eAB[ii // 2][:pi, (ii % 2) * 512 : (ii % 2 + 1) * 512]
                            nc.tensor.matmul(
                                py[:65],
                                V3[ii][:pi, h, 0:65],
                                src,
                                start=(ii == 0),
                                stop=(ii == NIC - 1),
                            )
                        # stash unnormalized y' and the Z row; free py quickly
                        j = h % 8
                        a, b = j % 4, j // 4
                        nc.vector.tensor_copy(YT[oc][hb : hb + 64, tsl], py[:64])
                        nc.vector.tensor_copy(
                            state["Zbuf"][32 * a : 32 * a + 1, 512 * b : 512 * b + 512],
                            py[64:65],
                        )
                        if h % 8 == 7:
                            Zr = zrp.tile(
                                [128, 1024], f32r, tag="zr", name=f"zr{h}"
                            )
                            with nc.allow_low_precision(reason="1/Z in f32r"):
                                nc.vector.reciprocal(Zr, state["Zbuf"])
                            state["Zr"][h // 8] = Zr
                        # batch-0 normalization spread over batch-1 iterations
                        if h >= 8:
                            norm_one(h - 8)

                    for h in range(H - 8, H):
                        norm_one(h)

            attention_half(0)

            # ---- x^T half 1 + Q^T projection half 1 ----
            xTh1 = [xth.tile([128, 512], f32r, tag=f"xTh{i}", name=f"xTh1_{i}") for i in range(NCC)]
            with tc.tile_pool(name="pt2", bufs=3, space="PSUM") as pt, \
                 tc.tile_pool(name="pa2", bufs=3, space="PSUM") as pa:
                for ts in range(4):
                    transpose_tiles(x_nat1[ts], xTh1, ts * 128, 128, pt)
                q_proj_half(1, xTh1, pa)

            attention_half(1)

        # ---- output projection ----
        with tc.tile_pool(name="osb", bufs=3) as osb, \
             tc.tile_pool(name="po", bufs=4, space="PSUM") as pos:
            bo_row = osb.tile([1, C], bf16, tag="bo_row")
            nc.sync.dma_start(out=bo_row, in_=bo_d[:].unsqueeze(0))
            for tt in range(8):
                ot = osb.tile([128, C], f32, tag="osb")
                for och in range(2):
                    po = pos.tile([128, 512], f32, tag="po")
                    for cc in range(NCC):
                        nc.tensor.matmul(
                            po,
                            YT[cc][:, tt * 128 : (tt + 1) * 128],
                            wo_t[cc][:, och * 512 : (och + 1) * 512],
                            start=(cc == 0),
                            stop=False,
                        )
                    nc.tensor.matmul(
                        po,
                        ones_bf[0:1, 0:128],
                        bo_row[0:1, och * 512 : (och + 1) * 512],
                        start=False,
                        stop=True,
                    )
                    if och == 0:
                        nc.scalar.copy(ot[:, 0:512], po)
                    else:
                        nc.vector.tensor_copy(ot[:, 512:1024], po)
                nc.sync.dma_start(out=out_d[tt * 128 : (tt + 1) * 128], in_=ot)

    nc.compile()
    return nc


def _get_nc():
    if "nc" not in _CACHE:
        _CACHE["nc"] = _build_nc()
    return _CACHE["nc"]


def _prep_in_maps(x, encoder_output, Wq, bq, Wkv, bkv, Wo, bo):
    import ml_dtypes

    f = np.float32
    x = np.asarray(x, f)
    enc = np.asarray(encoder_output, f)
    # fold softmax 1/sqrt(D) into Wq/bq
    wqT = np.ascontiguousarray(np.asarray(Wq, f).T * np.float32(SCALE))
    wkv = np.asarray(Wkv, f)
    wkT = np.ascontiguousarray(wkv[:C].T)
    wvT = np.ascontiguousarray(wkv[C:].T)
    woT = np.ascontiguousarray(np.asarray(Wo, f).T)
    bq = np.asarray(bq, f) * np.float32(SCALE)
    bkv = np.asarray(bkv, f)
    bo = np.asarray(bo, f)
    shared = {
        "wqT": wqT, "wkT": wkT, "wvT": wvT, "woT": woT,
        "bq": bq, "bk": np.ascontiguousarray(bkv[:C]),
        "bv": np.ascontiguousarray(bkv[C:]).astype(ml_dtypes.bfloat16),
        "bo": bo.astype(ml_dtypes.bfloat16),
    }
    return [
        dict(shared, x=np.ascontiguousarray(x[b]), enc=np.ascontiguousarray(enc[b]))
        for b in range(x.shape[0])
    ]


def kernel(x, encoder_output, Wq, bq, Wkv, bkv, Wo, bo):
    from concourse.bass_utils import run_bass_kernel_spmd

    nc = _get_nc()
    in_maps = _prep_in_maps(x, encoder_output, Wq, bq, Wkv, bkv, Wo, bo)
    res = run_bass_kernel_spmd(nc, in_maps, list(range(len(in_maps)))).results
    return np.stack([res[b]["out"] for b in range(len(res))]).astype(np.float32)


# revision 32
# speedup vs baseline: 1.6229x; 1.0686x over previous
"""Cross-attention kernel for Trainium2, 8 NeuronCores, data-parallel over batch.

Problem (per batch element b, one per core):
    q  = x_b @ Wq.T + bq                      [T=1024, C=1024]
    kv = enc_b @ Wkv.T + bkv                  [I=576, 2C]
    per head h (H=16, D=64):
        att = softmax((q_h @ k_h.T) / sqrt(D))
        y_h = att @ v_h
    out = y @ Wo.T + bo                       [T, C]

v2 design notes (vs baseline):
  - softmax 1/sqrt(D) folded into Wq/bq on host.
  - Weights DMA'd in [128, 1024] tiles (few, big transfers) ordered so the
    PE rarely waits; wq/wo reuse wk/wv buffer slots (sem-gated rotation).
  - Attention normalization uses reciprocal_approx_fast (DVE) and is
    pipelined one iteration deep: the rank-1 1/Z broadcast for iteration
    j-1 runs between iteration j's S^T and AV matmuls, so the PE never
    idles on the softmax denominator.
  - exp runs on 2-bank PSUM tiles ([*,1024] per ACT op) for i-chunks 0..3;
    the last (64-row) chunk uses a Schraudolph bit-trick exp on DVE
    (bits = round(s*128/ln2 + 16250.5) as int16, bitcast bf16).
  - V and exp(S) are bf16 (AV matmul bf16xbf16); Q/K logit path stays f32r.
  - Q/K bias adds moved to the ACT engine (per-partition bias), transpose
    copies alternate DVE/Pool, so no single engine gates the PE.
  - PE program order interleaves attention(t-half 0) between the two Q
    projection halves to spread ACT exp work.
"""

import numpy as np

T = 1024
C = 1024
I = 576
H = 16
D = 64
NCC = C // 128          # 8 contraction chunks
NIC = (I + 127) // 128  # 5 i chunks (128,128,128,128,64)
I_CH = [128, 128, 128, 128, 64]
VW = 66                 # per-head column block in V tile: 64 v cols + ones col + pad
SCALE = 1.0 / np.sqrt(D)
EXP_A16 = 128.0 / np.log(2.0)   # Schraudolph bf16 scale
EXP_B16 = 16256.0 - 5.5         # 127*2^7 minus tuned shift

_CACHE = {}


def _build_nc():
    import concourse.bass as bass
    import concourse.bacc as bacc
    import concourse.mybir as mybir
    import concourse.tile as tile
    from contextlib import ExitStack

    f32 = mybir.dt.float32
    f32r = mybir.dt.float32r
    bf16 = mybir.dt.bfloat16
    i16 = mybir.dt.int16

    nc = bacc.Bacc()

    x_d = nc.dram_tensor("x", [T, C], f32r, kind="ExternalInput")
    enc_d = nc.dram_tensor("enc", [I, C], f32r, kind="ExternalInput")
    wqT_d = nc.dram_tensor("wqT", [C, C], f32r, kind="ExternalInput")
    wkT_d = nc.dram_tensor("wkT", [C, C], f32r, kind="ExternalInput")
    wvT_d = nc.dram_tensor("wvT", [C, C], f32r, kind="ExternalInput")
    woT_d = nc.dram_tensor("woT", [C, C], f32r, kind="ExternalInput")
    bq_d = nc.dram_tensor("bq", [C], f32, kind="ExternalInput")
    bk_d = nc.dram_tensor("bk", [C], f32, kind="ExternalInput")
    bv_d = nc.dram_tensor("bv", [C], bf16, kind="ExternalInput")
    bo_d = nc.dram_tensor("bo", [C], bf16, kind="ExternalInput")
    out_d = nc.dram_tensor("out", [T, C], f32, kind="ExternalOutput")

    with ExitStack() as ctx:
        tc = ctx.enter_context(tile.TileContext(nc))

        # long-lived pools
        resid = ctx.enter_context(tc.tile_pool(name="resid", bufs=1))
        misc = ctx.enter_context(tc.tile_pool(name="misc", bufs=1))

        # constants
        ident_d = nc.inline_tensor(np.eye(128, dtype=np.float32), name="ident_d")
        _sel = np.zeros((128, 256), dtype=np.float32)
        for _r in range(4):
            _sel[32 * _r, _r * 64 : (_r + 1) * 64] = 1.0
        sel32_d = nc.inline_tensor(_sel, name="sel32_d")
        ident = misc.tile([128, 128], f32r)
        nc.sync.dma_start(out=ident, in_=ident_d[:, :].bitcast(f32r))
        sel32 = misc.tile([128, 256], f32r)
        nc.sync.dma_start(out=sel32, in_=sel32_d[:, :].bitcast(f32r))
        ones_bf = misc.tile([1, 128], bf16)
        nc.vector.memset(ones_bf, 1.0)
        bq_t = misc.tile([128, NCC], f32)
        nc.sync.dma_start(out=bq_t, in_=bq_d[:].rearrange("(oc p) -> p oc", p=128))
        bk_t = misc.tile([128, NCC], f32)
        nc.sync.dma_start(out=bk_t, in_=bk_d[:].rearrange("(oc p) -> p oc", p=128))
        bv_row = misc.tile([1, C], bf16)
        nc.sync.dma_start(out=bv_row, in_=bv_d[:].unsqueeze(0))

        # resident tensors
        QT = [resid.tile([128, T], f32r, tag=f"QT{i}", name=f"QT{i}") for i in range(NCC)]
        KT = [resid.tile([128, I], f32r, tag=f"KT{i}", name=f"KT{i}") for i in range(NCC)]
        V3 = [resid.tile([128, H, VW], bf16, tag=f"V{i}", name=f"V{i}") for i in range(NIC)]
        YT = [resid.tile([128, T], f32r, tag=f"YT{i}", name=f"YT{i}") for i in range(NCC)]

        copy_idx = [0]

        def transpose_tiles(src_tile, dst_list, dst_col0, pi, pt):
            """PE-transpose src_tile[:pi, cc-chunks] into dst_list[cc][:, col].
            PSUM->SBUF copies alternate DVE / Pool."""
            for cc in range(NCC):
                ps = pt.tile([128, 128], f32r, tag="pt")
                nc.tensor.transpose(
                    ps[:128, :pi],
                    src_tile[:pi, cc * 128 : (cc + 1) * 128],
                    ident[:pi, :pi],
                )
                if copy_idx[0] % 2 == 0:
                    nc.vector.tensor_copy(
                        dst_list[cc][:, dst_col0 : dst_col0 + pi], ps[:128, :pi]
                    )
                else:
                    nc.scalar.copy(
                        dst_list[cc][:, dst_col0 : dst_col0 + pi], ps[:128, :pi]
                    )
                copy_idx[0] += 1

        # SBUF pools live through phase A + attention; wst also through out-proj
        wst = ctx.enter_context(tc.tile_pool(name="wst", bufs=1))
        xin = ctx.enter_context(tc.tile_pool(name="xin", bufs=1))
        xth = ctx.enter_context(tc.tile_pool(name="xth", bufs=1))

        if True:

            # ---- DMA issue order (SP program order) ----
            enc_nat = []
            for ii in range(NIC):
                pi = I_CH[ii]
                e_nat = xin.tile([128, C], f32r, tag=f"xin{ii % 3}", name=f"enc_nat{ii}")
                nc.sync.dma_start(out=e_nat[:pi], in_=enc_d[ii * 128 : ii * 128 + pi])
                enc_nat.append(e_nat)
            x_nat0 = []
            for ts in range(4):
                xt = xin.tile([128, C], f32r, tag=f"xin{(ts + 1) % 3}", name=f"x_nat0_{ts}")
                nc.sync.dma_start(out=xt, in_=x_d[ts * 128 : (ts + 1) * 128])
                x_nat0.append(xt)
            wk_t = []
            for cc in range(NCC):
                w = wst.tile([128, C], f32r, tag=f"w{cc}", name=f"wk{cc}")
                nc.sync.dma_start(out=w, in_=wkT_d[cc * 128 : (cc + 1) * 128, :])
                wk_t.append(w)
            wv_t = []
            for cc in range(NCC):
                w = wst.tile([128, C], f32r, tag=f"w{8 + cc}", name=f"wv{cc}")
                nc.sync.dma_start(out=w, in_=wvT_d[cc * 128 : (cc + 1) * 128, :])
                wv_t.append(w)
            # wq reuses wk slots (gated on K proj), wo reuses wv slots (gated on V proj)
            wq_t = []
            for cc in range(NCC):
                w = wst.tile([128, C], f32r, tag=f"w{cc}", name=f"wq{cc}")
                nc.sync.dma_start(out=w, in_=wqT_d[cc * 128 : (cc + 1) * 128, :])
                wq_t.append(w)
            x_nat1 = []
            for ts in range(4):
                xt = xin.tile([128, C], f32r, tag=f"xin{(ts + 2) % 3}", name=f"x_nat1_{ts}")
                nc.sync.dma_start(out=xt, in_=x_d[(4 + ts) * 128 : (5 + ts) * 128])
                x_nat1.append(xt)
            wo_t = []
            for cc in range(NCC):
                w = wst.tile([128, C], f32r, tag=f"w{8 + cc}", name=f"wo{cc}")
                nc.sync.dma_start(out=w, in_=woT_d[cc * 128 : (cc + 1) * 128, :])
                wo_t.append(w)

            def q_proj_half(tch, xTh, pa):
                for oc in range(NCC):
                    pq = pa.tile([128, 512], f32, tag="pa")
                    for cc in range(NCC):
                        nc.tensor.matmul(
                            pq,
                            wq_t[cc][:, oc * 128 : (oc + 1) * 128],
                            xTh[cc],
                            start=(cc == 0),
                            stop=(cc == NCC - 1),
                        )
                    nc.scalar.add(
                        QT[oc][:, tch * 512 : (tch + 1) * 512],
                        pq,
                        bq_t[:, oc : oc + 1],
                    )

            xTh0 = [xth.tile([128, 512], f32r, tag=f"xTh{i}", name=f"xTh0_{i}") for i in range(NCC)]

            with tc.tile_pool(name="ph1", bufs=1) as ph1, \
                 tc.tile_pool(name="pt1", bufs=3, space="PSUM") as pt, \
                 tc.tile_pool(name="pa1", bufs=4, space="PSUM") as pa:

                encT = [ph1.tile([128, I], f32r, tag=f"encT{i}", name=f"encT{i}") for i in range(NCC)]

                # ---- enc^T, x^T half 0 ----
                for ii in range(NIC):
                    transpose_tiles(enc_nat[ii], encT, ii * 128, I_CH[ii], pt)
                for ts in range(4):
                    transpose_tiles(x_nat0[ts], xTh0, ts * 128, 128, pt)

                # ---- K^T projection (i in halves of 288); bias on ACT ----
                for oc in range(NCC):
                    pk = [pa.tile([128, 288], f32, tag="pa", name=f"pk{_}") for _ in range(2)]
                    for cc in range(NCC):
                        for ih in range(2):
                            nc.tensor.matmul(
                                pk[ih],
                                wk_t[cc][:, oc * 128 : (oc + 1) * 128],
                                encT[cc][:, ih * 288 : (ih + 1) * 288],
                                start=(cc == 0),
                                stop=(cc == NCC - 1),
                            )
                    for ih in range(2):
                        nc.scalar.add(
                            KT[oc][:, ih * 288 : (ih + 1) * 288],
                            pk[ih],
                            bk_t[:, oc : oc + 1],
                        )

                # ---- V projection into [128, H, VW] bf16 layout ----
                for ii in range(NIC):
                    nc.vector.memset(V3[ii][:, :, 64:65], 1.0)
                for och in range(2):
                    for ii in range(NIC):
                        pi = I_CH[ii]
                        pv = pa.tile([128, 512], f32, tag="pa")
                        for cc in range(NCC):
                            nc.tensor.matmul(
                                pv[:pi],
                                encT[cc][:, ii * 128 : ii * 128 + pi],
                                wv_t[cc][:, och * 512 : (och + 1) * 512],
                                start=(cc == 0),
                                stop=False,
                            )
                        nc.tensor.matmul(
                            pv[:pi],
                            ones_bf[0:1, :pi],
                            bv_row[0:1, och * 512 : (och + 1) * 512],
                            start=False,
                            stop=True,
                        )
                        dst = V3[ii][:pi, och * 8 : och * 8 + 8, 0:64]
                        nc.vector.tensor_copy(
                            dst, pv[:pi].rearrange("p (h d) -> p h d", d=64)
                        )

                # ---- Q^T projection half 0 ----
                q_proj_half(0, xTh0, pa)

            def attention_half(tch):
                # Normalization: Z rows are gathered into Zbuf[8, 512]; one
                # exact reciprocal per 8 heads (amortized); the 1/Z broadcast
                # (one-hot K=8 matmul) + in-place YT multiply run lagged by
                # one batch so the PE never waits on the reciprocal.
                with tc.tile_pool(name=f"es{tch}", bufs=4) as exps, \
                     tc.tile_pool(name=f"eS{tch}", bufs=3) as expss, \
                     tc.tile_pool(name=f"zb{tch}", bufs=1) as zbp, \
                     tc.tile_pool(name=f"zr{tch}", bufs=1) as zrp, \
                     tc.tile_pool(name=f"p2{tch}", bufs=2, space="PSUM") as pes2, \
                     tc.tile_pool(name=f"p1{tch}", bufs=1, space="PSUM") as pes1, \
                     tc.tile_pool(name=f"py{tch}", bufs=2, space="PSUM") as pys, \
                     tc.tile_pool(name=f"pz{tch}", bufs=1, space="PSUM") as pzs:
                    tsl = slice(tch * 512, (tch + 1) * 512)

                    # Z rows for head j=2b+a of a batch of 8 live at partition
                    # 32a, free block b (engine partition bases must be
                    # 32-aligned). Two batches per half.
                    # The loop is software-pipelined one head deep: AV(h-1)
                    # runs after S^T(h), so the exps of head h-1 have a full
                    # iteration of slack and the PE never waits on them.
                    state = {"Zbuf": None, "Zr": [None, None]}

                    def norm_one(h):
                        j = h % 8
                        a, b = j % 4, j // 4
                        oc_p, hb_p = h // 2, (h % 2) * 64
                        pb = pzs.tile([128, 512], f32, tag="pz")
                        nc.tensor.matmul(
                            pb[:64],
                            sel32[:, a * 64 : (a + 1) * 64],
                            state["Zr"][h // 8][:, 512 * b : 512 * b + 512],
                            start=True,
                            stop=True,
                        )
                        nc.vector.tensor_mul(
                            YT[oc_p][hb_p : hb_p + 64, tsl],
                            YT[oc_p][hb_p : hb_p + 64, tsl],
                            pb[:64],
                        )

                    def do_av(p):
                        h_p, eAB_p, eC_p = p
                        oc_p, hb_p = h_p // 2, (h_p % 2) * 64
                        py = pys.tile([128, 512], f32, tag="py", name=f"py{h_p}")
                        for ii in range(NIC):
                            pi = I_CH[ii]
                            src = eC_p[:pi, 0:512] if ii == 4 else \
                                eAB_p[ii // 2][:pi, (ii % 2) * 512 : (ii % 2 + 1) * 512]
                            nc.tensor.matmul(
                                py[:65],
                                V3[ii][:pi, h_p, 0:65],
                                src,
                                start=(ii == 0),
                                stop=(ii == NIC - 1),
                            )
                        # stash unnormalized y' and the Z row; free py quickly
                        j = h_p % 8
                        a, b = j % 4, j // 4
                        nc.vector.tensor_copy(
                            YT[oc_p][hb_p : hb_p + 64, tsl], py[:64]
                        )
                        nc.vector.tensor_copy(
                            state["Zbuf"][32 * a : 32 * a + 1, 512 * b : 512 * b + 512],
                            py[64:65],
                        )
                        if j == 7:
                            Zr = zrp.tile(
                                [128, 1024], f32r, tag="zr", name=f"zr{h_p}"
                            )
                            with nc.allow_low_precision(reason="1/Z in f32r"):
                                nc.vector.reciprocal(Zr, state["Zbuf"])
                            state["Zr"][h_p // 8] = Zr

                    pend = None
                    normed = 0
                    for h in range(H):
                        oc = h // 2
                        hb = (h % 2) * 64
                        if h % 8 == 0 and h < 8:
                            state["Zbuf"] = zbp.tile(
                                [128, 1024], f32, tag="zbuf", name=f"zbuf{h}"
                            )
                            nc.gpsimd.memset(state["Zbuf"], 1.0)
                        # S^T chunk pairs into 2-bank psum tiles
                        pAB = [pes2.tile([128, 1024], f32, tag="pe2", name=f"pAB{_}") for _ in range(2)]
                        pC = pes1.tile([128, 512], f32, tag="pe1")
                        for ii in range(NIC):
                            pi = I_CH[ii]
                            dst = pC[:pi, 0:512] if ii == 4 else \
                                pAB[ii // 2][:pi, (ii % 2) * 512 : (ii % 2 + 1) * 512]
                            nc.tensor.matmul(
                                dst,
                                KT[oc][hb : hb + 64, ii * 128 : ii * 128 + pi],
                                QT[oc][hb : hb + 64, tsl],
                                start=True,
                                stop=True,
                            )
                        # exp: chunks 0..3 exact on ACT, chunk 4 via
                        # Schraudolph bit-trick on DVE (bf16 out)
                        eAB = [exps.tile([128, 1024], bf16, tag="eL", name=f"eAB{_}") for _ in range(2)]
                        eC = expss.tile([128, 512], bf16, tag="eS")
                        for j in range(2):
                            for jh in range(2):
                                sl = slice(jh * 512, (jh + 1) * 512)
                                nc.scalar.activation(
                                    eAB[j][:, sl], pAB[j][:, sl],
                                    mybir.ActivationFunctionType.Exp,
                                )
                        with nc.allow_low_precision(reason="schraudolph exp, last chunk"):
                            nc.vector.tensor_scalar(
                                eC[:64].bitcast(i16),
                                pC[:64, 0:512],
                                float(EXP_A16),
                                float(EXP_B16),
                                mybir.AluOpType.mult,
                                mybir.AluOpType.add,
                            )
                        # batch-0 norms spread over later iterations (PE slot
                        # between S^T(h) and AV(h-1))
                        if h >= 10 and normed < 8:
                            norm_one(normed)
                            normed += 1
                        # AV for the previous head (its exps are long done)
                        if pend is not None:
                            do_av(pend)
                            if pend[0] % 8 == 7:
                                # new Zbuf for the batch now starting
                                state["Zbuf"] = zbp.tile(
                                    [128, 1024], f32, tag="zbuf", name=f"zbufn{h}"
                                )
                                nc.gpsimd.memset(state["Zbuf"], 1.0)
                        pend = (h, eAB, eC)

                    do_av(pend)
                    while normed < H:
                        norm_one(normed)
                        normed += 1

            attention_half(0)

            # ---- x^T half 1 + Q^T projection half 1 ----
            xTh1 = [xth.tile([128, 512], f32r, tag=f"xTh{i}", name=f"xTh1_{i}") for i in range(NCC)]
            with tc.tile_pool(name="pt2", bufs=3, space="PSUM") as pt, \
                 tc.tile_pool(name="pa2", bufs=3, space="PSUM") as pa:
                for ts in range(4):
                    transpose_tiles(x_nat1[ts], xTh1, ts * 128, 128, pt)
                q_proj_half(1, xTh1, pa)

            attention_half(1)

        # ---- output projection ----
        with tc.tile_pool(name="osb", bufs=3) as osb, \
             tc.tile_pool(name="po", bufs=4, space="PSUM") as pos:
            bo_row = osb.tile([1, C], bf16, tag="bo_row")
            nc.sync.dma_start(out=bo_row, in_=bo_d[:].unsqueeze(0))
            for tt in range(8):
                ot = osb.tile([128, C], f32, tag="osb")
                for och in range(2):
                    po = pos.tile([128, 512], f32, tag="po")
                    for cc in range(NCC):
                        nc.tensor.matmul(
                            po,
                            YT[cc][:, tt * 128 : (tt + 1) * 128],
                            wo_t[cc][:, och * 512 : (och + 1) * 512],
                            start=(cc == 0),
                            stop=False,
                        )
                    nc.tensor.matmul(
                        po,
                        ones_bf[0:1, 0:128],
                        bo_row[0:1, och * 512 : (och + 1) * 512],
                        start=False,
                        stop=True,
                    )
                    if och == 0:
                        nc.scalar.copy(ot[:, 0:512], po)
                    else:
                        nc.vector.tensor_copy(ot[:, 512:1024], po)
                nc.sync.dma_start(out=out_d[tt * 128 : (tt + 1) * 128], in_=ot)

    nc.compile()
    return nc


def _get_nc():
    if "nc" not in _CACHE:
        _CACHE["nc"] = _build_nc()
    return _CACHE["nc"]


def _prep_in_maps(x, encoder_output, Wq, bq, Wkv, bkv, Wo, bo):
    import ml_dtypes

    f = np.float32
    x = np.asarray(x, f)
    enc = np.asarray(encoder_output, f)
    # fold softmax 1/sqrt(D) into Wq/bq
    wqT = np.ascontiguousarray(np.asarray(Wq, f).T * np.float32(SCALE))
    wkv = np.asarray(Wkv, f)
    wkT = np.ascontiguousarray(wkv[:C].T)
    wvT = np.ascontiguousarray(wkv[C:].T)
    woT = np.ascontiguousarray(np.asarray(Wo, f).T)
    bq = np.asarray(bq, f) * np.float32(SCALE)
    bkv = np.asarray(bkv, f)
    bo = np.asarray(bo, f)
    shared = {
        "wqT": wqT, "wkT": wkT, "wvT": wvT, "woT": woT,
        "bq": bq, "bk": np.ascontiguousarray(bkv[:C]),
        "bv": np.ascontiguousarray(bkv[C:]).astype(ml_dtypes.bfloat16),
        "bo": bo.astype(ml_dtypes.bfloat16),
    }
    return [
        dict(shared, x=np.ascontiguousarray(x[b]), enc=np.ascontiguousarray(enc[b]))
        for b in range(x.shape[0])
    ]


def kernel(x, encoder_output, Wq, bq, Wkv, bkv, Wo, bo):
    from concourse.bass_utils import run_bass_kernel_spmd

    nc = _get_nc()
    in_maps = _prep_in_maps(x, encoder_output, Wq, bq, Wkv, bkv, Wo, bo)
    res = run_bass_kernel_spmd(nc, in_maps, list(range(len(in_maps)))).results
    return np.stack([res[b]["out"] for b in range(len(res))]).astype(np.float32)
